# revision 11
# baseline (speedup 1.0000x reference)
"""DCT-feature-extractor kernel for 8 Trainium2 NeuronCores.

Math collapse: the reference keeps only dct[0, 0:4] of each 8x8 block's 2-D
orthonormal-DFT real part.  Row 0 of the DFT matrix is constant (Fr[0,:] =
1/sqrt(8), Fi[0,:] = 0), so

    feat[m] = sum_l G[m, l] * colsum[l],   G[m, l] = cos(2*pi*m*l/8) / 8,

where colsum[l] is the column sum of the 8x8 block.  The whole module is then

    out[b, o] = sum_{i,j,m} W[o, (i*64+j)*4+m] * feat[b,i,j,m] + bias[o].

Sharding: split the 512 image rows (block-row groups i) and the matching
weight columns across 8 cores; the host sums the per-core partials + bias.

Everything streams in bf16 (host casts): tolerance is 2e-2 rel and bf16
keeps the error ~5e-3 while halving HBM traffic to ~4.2 MB/core.  All
descriptors are >=4KB per partition row -- smaller ones hit the ~150ns
per-descriptor service floor and halve effective DMA rate.

The host pre-TRANSPOSES the image shard so the in-block column index lands
on SBUF partitions: x[p=(j16,l8), f=(wg4, a8, i8, h2, b16)].  The 8-row
column sums become 3 contiguous-run adds over `a` (runs of 256 -> DVE 2x
eligible), split across DVE (wg 0,2) and GpSimd (wg 1,3).  The G matmul
consumes colsums directly -- no PE transposes.

Per-core schedule:
  SP ring:  x in 4 x 512KB wg-chunks, then the weight in 4 chunks (4KB
            rows) that the stage-3 matmuls chase.
  ACT ring: Gblk const load (first, so G-matmuls run during the x phase),
            final out store.
  PE:  4 G matmuls (column positions 0/64 per pft tile), then 16
       accumulating matmuls vs the reordered W^T shard spread over the 4 PE
       column groups (tile_position).
The 4 column-group partials are NOT collapsed on device: the kernel stores
the [128, 512] PSUM tile as bf16 and the host folds the 4 groups together
with the 8 per-core partials (one extra reshape-sum, off the critical
path).  The Bass entry barrier is stripped (it only guards unused
framework const memsets) so DMA descriptors issue immediately.
"""

import numpy as np
import ml_dtypes

import concourse.bacc as bacc
import concourse.mybir as mybir
from concourse.bass_utils import run_bass_kernel_spmd
from concourse.tile import TileContext

N_CORES = 8
B = 32            # batch
H = 512           # image height
WD = 512          # image width
BS = 8            # dct block size
NF = 4            # kept dct coefficients per block
OUT = 512         # linear output dim
RPC = H // N_CORES          # 64 rows per core
IPC = RPC // BS             # 8 block-rows per core
F32 = mybir.dt.float32
BF16 = mybir.dt.bfloat16
NPBF = ml_dtypes.bfloat16

NT = 2 * IPC      # 16 weight tiles of [128, 512]


def _g_mat():
    m = np.arange(NF)[:, None].astype(np.float64)
    l = np.arange(BS)[None, :].astype(np.float64)
    return (np.cos(2.0 * np.pi * m * l / BS) / 8.0).astype(np.float32)  # [4, 8]


def _consts():
    """Gblk [128, 64]: Gblk[p=(j16,l8), q=(j'16,m4)] = G[m, l] * (j16 == j'16).

    One matmul per w-group turns colsums [p=(j,l), f] into feats [q=(j,m), f].
    """
    g = _g_mat()
    c = np.zeros((128, 64), np.float32)
    for j in range(16):
        c[j * 8:(j + 1) * 8, j * 4:(j + 1) * 4] = g.T  # [l, m]
    return c


def _build_bass():
    nc = bacc.Bacc("TRN2", target_bir_lowering=False, debug=False)
    # Strip the Bass.__init__ entry barrier (drain + event-sem per engine):
    # it only guards framework const-AP memsets this kernel never reads, and
    # it stalls the DMA queues ~4us behind the slow-to-start Tensor engine.
    entry = nc.main_func.blocks[0]
    for inst in [
        i for i in entry.instructions
        if isinstance(i, (mybir.InstDrain, mybir.InstEventSemaphore))
    ]:
        entry.instructions.remove(inst)
    # x host-prepped: [p=(j16, l8), Gblk(64) | f=(wg4, a8, i8, h2, b16)]
    x = nc.dram_tensor("x", [128, 64 + 8192], BF16, kind="ExternalInput")
    # wt host-prepped: [p=(v2, j16, m4), t=(fi2, i8) x o512]
    wt = nc.dram_tensor("wt", [128, NT * OUT], BF16, kind="ExternalInput")
    out = nc.dram_tensor("out", [128, OUT], F32, kind="ExternalOutput")

    with TileContext(nc) as tc:
        with (
            tc.tile_pool(name="sb", bufs=1) as sb,
            tc.tile_pool(name="ps", bufs=1, space="PSUM") as ps,
        ):
            # ---- DMA program order == HWDGE FIFO order per ring ----
            # Both HWDGE rings stream in parallel (one descriptor generator
            # each -- a single ring is generation-limited and starves the
            # SDMA queues ~25%).  SP: x0, x2, wt(t0-3), wt(t8-11); ACT: x1,
            # x3, wt(t4-7), wt(t12-15).  Pairs land together, x first.
            # Gblk rides x chunk 0 -- no extra descriptors.
            xt = sb.tile([128, 64 + 8192], BF16, tag="x")
            wts = sb.tile([128, NT * OUT], BF16, tag="wt")
            for wg in range(4):
                lo = 0 if wg == 0 else 64 + wg * 2048
                eng = nc.sync if wg % 2 == 0 else nc.scalar
                eng.dma_start(
                    out=xt[:, lo:64 + (wg + 1) * 2048],
                    in_=x.ap()[:, lo:64 + (wg + 1) * 2048],
                )
            gblk = xt[:, 0:64]
            # SP: wt(t0-3) then wt(t12-15); ACT: wt(t4-7), wt(t8-11).
            # SP carries 0.25MB more than ACT, so its final transfer (the
            # last-consumed tiles) drains solo and its completion sem is not
            # delayed by a simultaneous transfer finishing on the other ring.
            for eng, k in [(nc.sync, 0), (nc.scalar, 1), (nc.scalar, 2),
                           (nc.sync, 3)]:
                eng.dma_start(
                    out=wts[:, k * 2048:(k + 1) * 2048],
                    in_=wt.ap()[:, k * 2048:(k + 1) * 2048],
                )

            # ---- stage 1+2 fused: feats = sum_a Gblk.T @ x[:, wg, a]
            # (8 accumulating PE matmuls per w-group; the a-sum rides the
            # contraction, so no elementwise adds anywhere) ----
            fts = []
            for half in range(2):
                pft = ps.tile([128, 256], F32, tag=f"pft{half}")
                for v in range(2):
                    wg = half * 2 + v
                    for a in range(8):
                        nc.tensor.matmul(
                            pft[v * 64:(v + 1) * 64, :],
                            gblk,
                            xt[:, 64 + wg * 2048 + a * 256:
                               64 + wg * 2048 + (a + 1) * 256],
                            start=(a == 0), stop=(a == 7),
                            tile_position=(0, v * 64),
                            skip_group_check=True,
                        )
                ft = sb.tile([128, 256], BF16, tag=f"ft{half}", name=f"ft{half}")
                nc.vector.tensor_copy(ft[:, :], pft[:, :])
                fts.append(ft)

            # ---- stage 3: 16 accumulating matmuls spread over the 4 PE
            # column groups (out partition offset 32*g -> tile_position), so
            # weight loads of one group overlap matmuls of another ----
            pout = ps.tile([128, OUT], F32, tag="pout")
            for t in range(NT):
                fi, i, g = t // 8, t % 8, t % 4
                nc.tensor.matmul(
                    pout[32 * g:32 * (g + 1), :],
                    fts[fi][:, 32 * i:32 * (i + 1)],
                    wts[:, t * OUT:(t + 1) * OUT],
                    start=(t < 4),
                    stop=(t >= NT - 4),
                    tile_position=(0, 32 * g),
                    skip_group_check=True,
                )
            # store the 4 col-group partials as bf16; host folds them.
            # Cast on ACT (fastest PSUM->SBUF path), then store partition
            # halves on both HWDGE rings in parallel.
            psb = sb.tile([128, OUT], F32, tag="psb")
            nc.scalar.copy(psb[:, :], pout[:, :])
            nc.sync.dma_start(out=out.ap()[0:64], in_=psb[0:64, :])
            nc.scalar.dma_start(out=out.ap()[64:128], in_=psb[64:128, :])

    nc.compile()
    return nc


_NC_CACHE = None


def _get_nc():
    global _NC_CACHE
    if _NC_CACHE is None:
        _NC_CACHE = _build_bass()
    return _NC_CACHE


_CST = np.ascontiguousarray(_consts().astype(NPBF))


def make_in_maps(imgs, weight):
    """Per-core input dicts: transposed channel-0 row slice + weight shard."""
    wr = weight.reshape(OUT, H // BS, WD // BS, NF)  # [o, i_glob, j, m]
    in_maps = []
    for c in range(N_CORES):
        xc = imgs[:, 0, RPC * c:RPC * (c + 1), :]    # [32, 64, 512]
        # [b=(h2,b16), row=(i8,a8), col=(wg4,j16,l8)] -> [(j,l), (wg,a,i,h,b16)]
        xd = xc.reshape(2, 16, IPC, BS, 4, 16, BS).transpose(5, 6, 4, 3, 2, 0, 1)
        xd = np.concatenate([_CST, xd.reshape(128, 8192).astype(NPBF)], axis=1)
        xd = np.ascontiguousarray(xd)
        wc = wr[:, IPC * c:IPC * (c + 1)]            # [o, i, j, m]
        # p = v*64 + j16*4 + m  (j = (fi*2 + v)*16 + j16),  t = fi*8 + i
        wtc = wc.reshape(OUT, IPC, 2, 2, 16, NF)     # o, i, fi, v, j16, m
        wtc = wtc.transpose(3, 4, 5, 2, 1, 0)        # v, j16, m, fi, i, o
        wtc = np.ascontiguousarray(wtc.reshape(128, NT * OUT).astype(NPBF))
        in_maps.append({"x": xd, "wt": wtc})
    return in_maps


def kernel(imgs_tensors, weight, bias, block_size=8, num_features=4, **_):
    assert int(block_size) == BS and int(num_features) == NF
    imgs = np.ascontiguousarray(np.asarray(imgs_tensors, dtype=np.float32))
    w = np.ascontiguousarray(np.asarray(weight, dtype=np.float32))
    b = np.asarray(bias, dtype=np.float32)
    assert imgs.shape == (B, 3, H, WD) and w.shape == (OUT, H // BS * WD // BS * NF)

    nc = _get_nc()
    res = run_bass_kernel_spmd(nc, make_in_maps(imgs, w), core_ids=list(range(N_CORES)))
    acc = np.zeros((B, OUT), np.float32)
    for r in res.results:
        acc += r["out"].reshape(4, B, OUT).sum(axis=0)
    return (acc + b[None, :]).astype(np.float32)


# revision 12
# speedup vs baseline: 1.0984x; 1.0984x over previous
"""DCT-feature-extractor kernel for 8 Trainium2 NeuronCores.

Math collapse: the reference keeps only dct[0, 0:4] of each 8x8 block's 2-D
orthonormal-DFT real part.  Row 0 of the DFT matrix is constant (Fr[0,:] =
1/sqrt(8), Fi[0,:] = 0), so

    feat[m] = sum_l G[m, l] * colsum[l],   G[m, l] = cos(2*pi*m*l/8) / 8,

where colsum[l] is the column sum of the 8x8 block.  The whole module is then

    out[b, o] = sum_{i,j,m} W[o, (i*64+j)*4+m] * feat[b,i,j,m] + bias[o].

Sharding: split the 512 image rows (block-row groups i) and the matching
weight columns across 8 cores; the host sums the per-core partials + bias.

Everything streams in bf16 (host casts): tolerance is 2e-2 rel and bf16
keeps the error ~5e-3 while halving HBM traffic to ~4.2 MB/core.  All
descriptors are >=4KB per partition row -- smaller ones hit the ~150ns
per-descriptor service floor and halve effective DMA rate.

The host pre-TRANSPOSES the image shard so the in-block column index lands
on SBUF partitions: x[p=(j16,l8), f=(wg4, a8, i8, h2, b16)].  The 8-row
column sums become 3 contiguous-run adds over `a` (runs of 256 -> DVE 2x
eligible), split across DVE (wg 0,2) and GpSimd (wg 1,3).  The G matmul
consumes colsums directly -- no PE transposes.

Per-core schedule:
  SP ring:  x in 4 x 512KB wg-chunks, then the weight in 4 chunks (4KB
            rows) that the stage-3 matmuls chase.
  ACT ring: Gblk const load (first, so G-matmuls run during the x phase),
            final out store.
  PE:  4 G matmuls (column positions 0/64 per pft tile), then 16
       accumulating matmuls vs the reordered W^T shard spread over the 4 PE
       column groups (tile_position).
The 4 column-group partials are NOT collapsed on device: the kernel stores
the [128, 512] PSUM tile as bf16 and the host folds the 4 groups together
with the 8 per-core partials (one extra reshape-sum, off the critical
path).  The Bass entry barrier is stripped (it only guards unused
framework const memsets) so DMA descriptors issue immediately.
"""

import numpy as np
import ml_dtypes

import concourse.bacc as bacc
import concourse.mybir as mybir
from concourse.bass_utils import run_bass_kernel_spmd
from concourse.tile import TileContext

N_CORES = 8
B = 32            # batch
H = 512           # image height
WD = 512          # image width
BS = 8            # dct block size
NF = 4            # kept dct coefficients per block
OUT = 512         # linear output dim
RPC = H // N_CORES          # 64 rows per core
IPC = RPC // BS             # 8 block-rows per core
F32 = mybir.dt.float32
BF16 = mybir.dt.bfloat16
NPBF = ml_dtypes.bfloat16

NT = 2 * IPC      # 16 weight tiles of [128, 512]


def _g_mat():
    m = np.arange(NF)[:, None].astype(np.float64)
    l = np.arange(BS)[None, :].astype(np.float64)
    return (np.cos(2.0 * np.pi * m * l / BS) / 8.0).astype(np.float32)  # [4, 8]


def _consts():
    """Gblk [128, 64]: Gblk[p=(j16,l8), q=(j'16,m4)] = G[m, l] * (j16 == j'16).

    One matmul per w-group turns colsums [p=(j,l), f] into feats [q=(j,m), f].
    """
    g = _g_mat()
    c = np.zeros((128, 64), np.float32)
    for j in range(16):
        c[j * 8:(j + 1) * 8, j * 4:(j + 1) * 4] = g.T  # [l, m]
    return c


def _build_bass():
    nc = bacc.Bacc("TRN2", target_bir_lowering=False, debug=False)
    # Strip the Bass.__init__ entry barrier (drain + event-sem per engine):
    # it only guards framework const-AP memsets this kernel never reads, and
    # it stalls the DMA queues ~4us behind the slow-to-start Tensor engine.
    entry = nc.main_func.blocks[0]
    for inst in [
        i for i in entry.instructions
        if isinstance(i, (mybir.InstDrain, mybir.InstEventSemaphore))
    ]:
        entry.instructions.remove(inst)
    # x host-prepped: [p=(j16, l8), Gblk(64) | f=(wg4, a8, i8, h2, b16)]
    x = nc.dram_tensor("x", [128, 64 + 8192], BF16, kind="ExternalInput")
    # wt host-prepped: [p=(v2, j16, m4), t=(fi2, i8) x o512]
    wt = nc.dram_tensor("wt", [128, NT * OUT], BF16, kind="ExternalInput")
    out = nc.dram_tensor("out", [128, OUT], F32, kind="ExternalOutput")

    with TileContext(nc) as tc:
        with (
            tc.tile_pool(name="sb", bufs=1) as sb,
            tc.tile_pool(name="ps", bufs=1, space="PSUM") as ps,
        ):
            # ---- DMA program order == HWDGE FIFO order per ring ----
            # Both HWDGE rings stream in parallel (one descriptor generator
            # each -- a single ring is generation-limited and starves the
            # SDMA queues ~25%).  SP: x0, x2, wt(t0-3), wt(t8-11); ACT: x1,
            # x3, wt(t4-7), wt(t12-15).  Pairs land together, x first.
            # Gblk rides x chunk 0 -- no extra descriptors.
            xt = sb.tile([128, 64 + 8192], BF16, tag="x")
            wts = sb.tile([128, NT * OUT], BF16, tag="wt")
            for wg in range(4):
                lo = 0 if wg == 0 else 64 + wg * 2048
                eng = nc.sync if wg % 2 == 0 else nc.scalar
                eng.dma_start(
                    out=xt[:, lo:64 + (wg + 1) * 2048],
                    in_=x.ap()[:, lo:64 + (wg + 1) * 2048],
                )
            gblk = xt[:, 0:64]
            # SP: wt(t0-3) then wt(t12-15); ACT: wt(t4-7), wt(t8-11).
            # SP carries 0.25MB more than ACT, so its final transfer (the
            # last-consumed tiles) drains solo and its completion sem is not
            # delayed by a simultaneous transfer finishing on the other ring.
            for eng, k in [(nc.sync, 0), (nc.scalar, 1), (nc.scalar, 2),
                           (nc.sync, 3)]:
                eng.dma_start(
                    out=wts[:, k * 2048:(k + 1) * 2048],
                    in_=wt.ap()[:, k * 2048:(k + 1) * 2048],
                )

            # ---- stage 1+2 fused: feats = sum_a Gblk.T @ x[:, wg, a]
            # (8 accumulating PE matmuls per w-group; the a-sum rides the
            # contraction, so no elementwise adds anywhere) ----
            fts = []
            for half in range(2):
                pft = ps.tile([128, 256], F32, tag=f"pft{half}")
                for v in range(2):
                    wg = half * 2 + v
                    for a in range(8):
                        nc.tensor.matmul(
                            pft[v * 64:(v + 1) * 64, :],
                            gblk,
                            xt[:, 64 + wg * 2048 + a * 256:
                               64 + wg * 2048 + (a + 1) * 256],
                            start=(a == 0), stop=(a == 7),
                            tile_position=(0, v * 64),
                            skip_group_check=True,
                        )
                ft = sb.tile([128, 256], BF16, tag=f"ft{half}", name=f"ft{half}")
                nc.vector.tensor_copy(ft[:, :], pft[:, :])
                fts.append(ft)

            # ---- stage 3: 16 accumulating matmuls spread over the 4 PE
            # column groups (out partition offset 32*g -> tile_position), so
            # weight loads of one group overlap matmuls of another ----
            pout = ps.tile([128, OUT], F32, tag="pout")
            for t in range(NT):
                fi, i, g = t // 8, t % 8, t % 4
                nc.tensor.matmul(
                    pout[32 * g:32 * (g + 1), :],
                    fts[fi][:, 32 * i:32 * (i + 1)],
                    wts[:, t * OUT:(t + 1) * OUT],
                    start=(t < 4),
                    stop=(t >= NT - 4),
                    tile_position=(0, 32 * g),
                    skip_group_check=True,
                )
            # store the 4 col-group partials as bf16; host folds them.
            # Cast on ACT (fastest PSUM->SBUF path), then store partition
            # halves on both HWDGE rings in parallel.
            psb = sb.tile([128, OUT], F32, tag="psb")
            nc.scalar.copy(psb[:, :], pout[:, :])
            nc.scalar.dma_start(out=out.ap(), in_=psb[:, :])

    nc.compile()
    return nc


_NC_CACHE = None


def _get_nc():
    global _NC_CACHE
    if _NC_CACHE is None:
        _NC_CACHE = _build_bass()
    return _NC_CACHE


_CST = np.ascontiguousarray(_consts().astype(NPBF))


def make_in_maps(imgs, weight):
    """Per-core input dicts: transposed channel-0 row slice + weight shard."""
    wr = weight.reshape(OUT, H // BS, WD // BS, NF)  # [o, i_glob, j, m]
    in_maps = []
    for c in range(N_CORES):
        xc = imgs[:, 0, RPC * c:RPC * (c + 1), :]    # [32, 64, 512]
        # [b=(h2,b16), row=(i8,a8), col=(wg4,j16,l8)] -> [(j,l), (wg,a,i,h,b16)]
        xd = xc.reshape(2, 16, IPC, BS, 4, 16, BS).transpose(5, 6, 4, 3, 2, 0, 1)
        xd = np.concatenate([_CST, xd.reshape(128, 8192).astype(NPBF)], axis=1)
        xd = np.ascontiguousarray(xd)
        wc = wr[:, IPC * c:IPC * (c + 1)]            # [o, i, j, m]
        # p = v*64 + j16*4 + m  (j = (fi*2 + v)*16 + j16),  t = fi*8 + i
        wtc = wc.reshape(OUT, IPC, 2, 2, 16, NF)     # o, i, fi, v, j16, m
        wtc = wtc.transpose(3, 4, 5, 2, 1, 0)        # v, j16, m, fi, i, o
        wtc = np.ascontiguousarray(wtc.reshape(128, NT * OUT).astype(NPBF))
        in_maps.append({"x": xd, "wt": wtc})
    return in_maps


def kernel(imgs_tensors, weight, bias, block_size=8, num_features=4, **_):
    assert int(block_size) == BS and int(num_features) == NF
    imgs = np.ascontiguousarray(np.asarray(imgs_tensors, dtype=np.float32))
    w = np.ascontiguousarray(np.asarray(weight, dtype=np.float32))
    b = np.asarray(bias, dtype=np.float32)
    assert imgs.shape == (B, 3, H, WD) and w.shape == (OUT, H // BS * WD // BS * NF)

    nc = _get_nc()
    res = run_bass_kernel_spmd(nc, make_in_maps(imgs, w), core_ids=list(range(N_CORES)))
    acc = np.zeros((B, OUT), np.float32)
    for r in res.results:
        acc += r["out"].reshape(4, B, OUT).sum(axis=0)
    return (acc + b[None, :]).astype(np.float32)
